# revision 41
# baseline (speedup 1.0000x reference)
"""Bidirectional LSTM + attention pooling on 8 Trainium2 NeuronCores.

Strategy (direction-split data parallel):
  cores 0-3: forward LSTM, sample shards of 32;  cores 4-7: backward LSTM,
  same shards with host-side time reversal. One SPMD program; every per-core
  asymmetry (direction, shard, time remap, score-exchange permutation) is
  carried in the input data, never in control flow.

Per core: gather+tfidf-scale embeddings (window-pipelined), transpose to
[E, tokens], input-projection matmuls accumulate gate preactivations for an
8-step window directly in PSUM; the recurrence then adds Whh.T-stationary
matmuls per step and runs the gate nonlinearities in the transposed layout
([gate-dim partitions, batch free], so ACT/DVE cost scales with batch=16/32
free elems, not 1024). Attention scores accumulate online into a PSUM bank
via K=1 matmuls. The forward/backward partial scores are exchanged with a
pair AllGather; time alignment is done by an indirect-DMA row gather with a
per-core permutation input. Softmax (no max-subtraction needed: |tanh|<=1),
attention pooling via per-sample matmuls over PE-transposed h chunks, and the
output projection all run on-device; the host sums the fwd/bwd partial [3,32]
outputs.

The per-core batch of 32 is processed as two ping-pong groups of 16 so one
group's PE matmuls overlap the other group's ACT/DVE gate chain.
"""
import numpy as np

import concourse.bass as bass
import concourse.mybir as mybir
import concourse.tile as tile
from concourse import bacc
from concourse.bass_utils import run_bass_kernel_spmd
from concourse.masks import make_identity

F16 = mybir.dt.float16
F32 = mybir.dt.float32
I32 = mybir.dt.int32
AF = mybir.ActivationFunctionType
OP = mybir.AluOpType

V, E, H, OUT = 50000, 300, 256, 3
T = 512          # time steps (configurable for small tests)
BS = 32          # samples per core
NG = 2           # ping-pong groups
GB = BS // NG    # 16 samples per group
W = 8            # window steps (proj/gather granularity)
H4 = 4 * H       # 1024
NMT = H4 // 128  # 8 gate M-tiles
EK = [128, 128, 44]  # E chunks
ECH = len(EK)

_NC_CACHE = {}


def build(t_steps=T, debug=False):
    nw = t_steps // W              # windows
    ntb = t_steps * BS             # tokens per core
    ngt = ntb // 128               # gather tiles (128 tokens each)
    gpw = ngt // nw                # gather tiles per window (2 at BS=32, W=8)
    nsc = t_steps // 128           # 128-step chunks (scores/bridge/mask)
    assert t_steps % 128 == 0 and ngt % nw == 0

    nc = bacc.Bacc("TRN2", target_bir_lowering=False, debug=False,
                   enable_asserts=False, num_devices=8)

    def din(name, shape, dt):
        return nc.dram_tensor(name, shape, dt, kind="ExternalInput").ap()

    emb = din("emb", [V, E], F32)
    idx_sw = din("idx_sw", [128, ngt], I32)       # p-major swizzled token ids
    wtok_sw = din("wtok_sw", [128, ngt], F32)     # tfidf/denom, same order
    maskT = din("maskT", [t_steps, GB * NG], F32)  # [s, b]
    wihT = din("wihT", [E, H4], F16)
    whhT = din("whhT", [H, H4], F16)
    # bias2[st, gi*128+p] = (bih+bhh)[gi*256+st*128+p]
    bias2 = din("bias2", [2, H4 // 2], F16)
    sel2_in = din("sel2_in", [2, 2 * W * BS], F16)
    watt = din("watt", [128, 2], F16)             # w_att half, 2 chunks of 128
    battb = din("battb", [128, 1], F32)           # b_att (fwd) / 0 (bwd)
    maskwin = din("maskwin", [1, t_steps * BS], F16)  # s-major mask row
    permidx = din("permidx", [128, nsc], I32)     # cc-gather rows
    wotT = din("wotT", [H, OUT], F16)             # W_out half, transposed
    boutc = din("boutc", [OUT, 1], F32)           # b_out (fwd) / 0 (bwd)

    outp = nc.dram_tensor("outp", [OUT, BS], F32, kind="ExternalOutput").ap()
    if debug:
        dbg_hst = nc.dram_tensor("dbg_hst", [128, 2 * BS * t_steps], F16,
                                 kind="ExternalOutput").ap()
        dbg_sT = nc.dram_tensor("dbg_sT", [128, (t_steps // 128) * BS], F32,
                                kind="ExternalOutput").ap()
        dbg_att = nc.dram_tensor("dbg_att", [128, (t_steps // 128) * BS], F16,
                                 kind="ExternalOutput").ap()
        dbg_rep = nc.dram_tensor("dbg_rep", [128, 2 * BS], F16,
                                 kind="ExternalOutput").ap()
        dbg_ownT = nc.dram_tensor("dbg_ownT", [128, (t_steps // 128) * BS],
                                  F32, kind="ExternalOutput").ap()
        dbg_other = nc.dram_tensor("dbg_other", [128, (t_steps // 128) * BS],
                                   F32, kind="ExternalOutput").ap()
        dbg_gpre = nc.dram_tensor("dbg_gpre", [128, 4 * 2 * W * BS], F32,
                                  kind="ExternalOutput").ap()
        dbg_mbcw = nc.dram_tensor("dbg_mbcw", [128, W * BS], F32,
                                  kind="ExternalOutput").ap()
        dbg_embT = nc.dram_tensor("dbg_embT", [128, ECH * W * BS], F16,
                                  kind="ExternalOutput").ap()

    cc_in = nc.dram_tensor("cc_in", [t_steps, BS], F32, kind="Internal").ap()
    cc_gather = nc.dram_tensor("cc_gather", [2 * t_steps, BS], F32,
                               kind="Internal").ap()

    with tile.TileContext(nc) as tc:
        with tc.tile_pool(name="const", bufs=1) as cp, \
             tc.tile_pool(name="work", bufs=1) as wp, \
             tc.tile_pool(name="gath", bufs=4) as gp, \
             tc.tile_pool(name="embw", bufs=2) as ep, \
             tc.tile_pool(name="psg", bufs=1, space="PSUM") as psg, \
             tc.tile_pool(name="pst", bufs=3, space="PSUM") as pst:

            def pscratch(shape, dt):
                return pst.tile(shape, dt, name="pscratch", tag="pscratch")

            # ---------- constant / persistent tiles ----------
            wihT_sb = cp.tile([128, ECH * H4], F16)
            for c in range(ECH):
                nc.sync.dma_start(out=wihT_sb[0:EK[c], c * H4:(c + 1) * H4],
                                  in_=wihT[c * 128:c * 128 + EK[c], :])
            whhT_sb = cp.tile([128, 2 * H4], F16)
            for c in range(2):
                nc.sync.dma_start(out=whhT_sb[:, c * H4:(c + 1) * H4],
                                  in_=whhT[c * 128:(c + 1) * 128, :])
            bias2_sb = cp.tile([2, H4 // 2], F16)
            nc.sync.dma_start(out=bias2_sb[:], in_=bias2[:])
            watt_sb = cp.tile([128, 2], F16)
            nc.sync.dma_start(out=watt_sb[:], in_=watt[:])
            battb_sb = cp.tile([128, 1], F32)
            nc.sync.dma_start(out=battb_sb[:], in_=battb[:])
            idx_sb = cp.tile([128, ngt], I32)
            nc.sync.dma_start(out=idx_sb[:], in_=idx_sw[:])
            wtok_sb = cp.tile([128, ngt], F32)
            nc.sync.dma_start(out=wtok_sb[:], in_=wtok_sw[:])
            maskT_sb = cp.tile([128, nsc * BS], F32)
            for c in range(nsc):
                nc.sync.dma_start(out=maskT_sb[:, c * BS:(c + 1) * BS],
                                  in_=maskT[c * 128:(c + 1) * 128, :])
            permidx_sb = cp.tile([128, nsc], I32)
            nc.sync.dma_start(out=permidx_sb[:], in_=permidx[:])
            wotT_sb = cp.tile([128, 2 * OUT], F16)
            for c in range(2):
                nc.sync.dma_start(out=wotT_sb[:, c * OUT:(c + 1) * OUT],
                                  in_=wotT[c * 128:(c + 1) * 128, :])
            boutc_sb = cp.tile([OUT, 1], F32)
            nc.sync.dma_start(out=boutc_sb[:], in_=boutc[:])

            ident32 = cp.tile([128, 128], F32)
            make_identity(nc, ident32[:])
            ident16 = cp.tile([128, 128], F16)
            nc.vector.tensor_copy(out=ident16[:], in_=ident32[:])
            ones16 = cp.tile([128, 1], F16)
            nc.gpsimd.memset(ones16[:], 1.0)
            onesrow = cp.tile([1, max(512, W * BS)], F16)
            nc.gpsimd.memset(onesrow[:], 1.0)
            zerorow = cp.tile([1, 128], F16)
            nc.gpsimd.memset(zerorow[:], 0.0)
            # sel2[k, st*W*BS+j] = (k == st): routes bias strips to bank halves
            sel2 = cp.tile([2, 2 * W * BS], F16)
            nc.sync.dma_start(out=sel2[:], in_=sel2_in[:])
            maskwin_sb = cp.tile([1, t_steps * BS], F16)
            nc.sync.dma_start(out=maskwin_sb[:], in_=maskwin[:])

            # h store: [128, strip(2) * b(BS) * s(t_steps)] fp16, col = st*BS*T + b*T + s
            hstore = cp.tile([128, 2 * BS * t_steps], F16)
            # carries: [128, group, strip, GB]
            hcar = cp.tile([128, NG * 2 * GB], F16)
            ccar = cp.tile([128, NG * 2 * GB], F32)
            nc.gpsimd.memset(hcar[:], 0.0)
            nc.gpsimd.memset(ccar[:], 0.0)

            # gate-preactivation PSUM tiles: one per gate, [128, 2*W*BS] f32
            # col = strip_local*(W*BS) + sl*BS + g*GB + b
            pw = [psg.tile([128, 2 * W * BS], F32, name=f"pw{gi}", tag=f"pw{gi}")
                  for gi in range(4)]
            # group g rows at base partition 32*g (matmul base must be 0/32/64)
            score_ps = psg.tile([64, t_steps], F32, name="score_ps",
                                tag="score_ps")
            # one full-bank start=True zero-fill; all score MMs accumulate
            nc.tensor.matmul(out=score_ps[:], lhsT=zerorow[:, 0:64],
                             rhs=onesrow[:, 0:t_steps], start=True, stop=False)

            # working gate tiles (per group): [128, 2*GB]
            def gtile(nm):
                return wp.tile([128, 2 * GB], F32, name=nm, tag=nm, bufs=2)

            # ---------- main loop over windows ----------
            for w in range(nw):
                embT = ep.tile([128, ECH * W * BS], F16, tag="embT")
                for g in range(gpw):
                    gt = w * gpw + g
                    embg = gp.tile([128, E], F32, tag="embg")
                    nc.gpsimd.indirect_dma_start(
                        out=embg[:], out_offset=None, in_=emb[:],
                        in_offset=bass.IndirectOffsetOnAxis(
                            ap=idx_sb[:, gt:gt + 1], axis=0))
                    emb_sc = gp.tile([128, E], F16, tag="emb_sc")
                    nc.vector.tensor_scalar(
                        out=emb_sc[:], in0=embg[:],
                        scalar1=wtok_sb[:, gt:gt + 1], scalar2=None,
                        op0=OP.mult)
                    etp = pscratch([128, ECH * 128], F16)
                    for c in range(ECH):
                        nc.tensor.transpose(
                            out=etp[0:EK[c], c * 128:(c + 1) * 128],
                            in_=emb_sc[:, c * 128:c * 128 + EK[c]],
                            identity=ident16[:])
                    # one strided copy: [128, ECH, 128] -> embT cols c*W*BS + g*128
                    nc.scalar.activation(
                        out=embT[:].rearrange(
                            "p (c x) -> p c x", c=ECH)[
                            :, :, g * 128:(g + 1) * 128],
                        in_=etp[:].rearrange("p (c x) -> p c x", c=ECH),
                        func=AF.Copy)

                # projection into gate psum for this window.
                # Exactly ONE start=True matmul per bank per window (start
                # clears has_written bank-wide): the K=2 bias fill covers the
                # full bank, then everything accumulates in any order.
                for gi in range(4):
                    nc.tensor.matmul(
                        out=pw[gi][:],
                        lhsT=bias2_sb[:, gi * 128:(gi + 1) * 128],
                        rhs=sel2[:], start=True, stop=False)
                for mt in range(NMT):
                    gi, sl_ = mt // 2, mt % 2
                    dst = pw[gi][:, sl_ * (W * BS):(sl_ + 1) * (W * BS)]
                    for c in range(ECH):
                        nc.tensor.matmul(
                            out=dst,
                            lhsT=wihT_sb[0:EK[c], c * H4 + mt * 128:
                                         c * H4 + (mt + 1) * 128],
                            rhs=embT[0:EK[c], c * W * BS:(c + 1) * W * BS],
                            start=False, stop=False)

                # mask broadcast for the whole window: [128, W*BS] psum
                mbcw = pscratch([128, W * BS], F32)
                nc.tensor.matmul(
                    out=mbcw[:], lhsT=onesrow[:, 0:128],
                    rhs=maskwin_sb[:, w * W * BS:(w + 1) * W * BS],
                    start=True, stop=True)
                mbcwi = wp.tile([128, W * BS], I32, name="mbcwi",
                                tag="mbcwi", bufs=2)
                nc.vector.tensor_copy(out=mbcwi[:], in_=mbcw[:])

                if debug and w == 0:
                    dbg_embT_sb = wp.tile([128, ECH * W * BS], F16)
                    nc.vector.tensor_copy(out=dbg_embT_sb[:], in_=embT[:])
                    nc.sync.dma_start(out=dbg_embT[:], in_=dbg_embT_sb[:])
                    dbg_mbcw_sb = wp.tile([128, W * BS], F32)
                    nc.vector.tensor_copy(out=dbg_mbcw_sb[:], in_=mbcw[:])
                    nc.sync.dma_start(out=dbg_mbcw[:], in_=dbg_mbcw_sb[:])

                # ---------- steps in window ----------
                for sl in range(W):
                    s = w * W + sl
                    if debug and w == 0 and sl == W - 1:
                        # dump gate psums after ALL this window's rec-MMs
                        # (scheduled by deps after sl loop emits; reads full
                        # window region so it waits for every writer)
                        dbg_gpre_sb = wp.tile([128, 4 * 2 * W * BS], F32)
                        for gi in range(4):
                            nc.scalar.activation(
                                out=dbg_gpre_sb[:, gi * 2 * W * BS:
                                                (gi + 1) * 2 * W * BS],
                                in_=pw[gi][:], func=AF.Copy)
                        nc.sync.dma_start(out=dbg_gpre[:], in_=dbg_gpre_sb[:])
                    for g in range(NG):
                        gofs = g * GB
                        # recurrent matmuls: accumulate into psum col slice
                        for mt in range(NMT):
                            gi, sl_ = mt // 2, mt % 2
                            dst = pw[gi][:, sl_ * W * BS + sl * BS + gofs:
                                         sl_ * W * BS + sl * BS + gofs + GB]
                            for kc in range(2):
                                nc.tensor.matmul(
                                    out=dst,
                                    lhsT=whhT_sb[:, kc * H4 + mt * 128:
                                                 kc * H4 + (mt + 1) * 128],
                                    rhs=hcar[:, (g * 2 + kc) * GB:
                                             (g * 2 + kc + 1) * GB],
                                    start=False, stop=(kc == 1))

                        # gate AP for (sl, g): [128, 2strips, GB]
                        def gap(gi):
                            return pw[gi][:].rearrange(
                                "p (st x) -> p st x", st=2)[
                                :, :, sl * BS + gofs: sl * BS + gofs + GB]

                        sig_i = gtile("sig_i")
                        sig_f = gtile("sig_f")
                        tau_g = gtile("tau_g")
                        sig_o = gtile("sig_o")
                        si = sig_i[:].rearrange("p (st x) -> p st x", st=2)
                        sf = sig_f[:].rearrange("p (st x) -> p st x", st=2)
                        tg = tau_g[:].rearrange("p (st x) -> p st x", st=2)
                        so = sig_o[:].rearrange("p (st x) -> p st x", st=2)
                        nc.scalar.activation(out=si, in_=gap(0), func=AF.Sigmoid)
                        nc.scalar.activation(out=sf, in_=gap(1), func=AF.Sigmoid)
                        nc.scalar.activation(out=tg, in_=gap(2), func=AF.Tanh)
                        nc.scalar.activation(out=so, in_=gap(3), func=AF.Sigmoid)

                        cg = ccar[:, g * 2 * GB:(g + 1) * 2 * GB]
                        hg = hcar[:, g * 2 * GB:(g + 1) * 2 * GB]
                        cn = gtile("cn")
                        t2 = gtile("t2")
                        nc.vector.tensor_tensor(out=cn[:], in0=sig_f[:],
                                                in1=cg, op=OP.mult)
                        nc.vector.tensor_tensor(out=t2[:], in0=sig_i[:],
                                                in1=tau_g[:], op=OP.mult)
                        nc.vector.tensor_tensor(out=cn[:], in0=cn[:],
                                                in1=t2[:], op=OP.add)
                        mb = mbcw[:, sl * BS + gofs: sl * BS + gofs + GB][
                            :, None, :].to_broadcast([128, 2, GB])
                        mbi = mbcwi[:, sl * BS + gofs: sl * BS + gofs + GB][
                            :, None, :].to_broadcast([128, 2, GB])
                        # predicated update of carry c with c_new where valid
                        nc.vector.copy_predicated(
                            out=cg.rearrange("p (st y) -> p st y", st=2),
                            mask=mbi,
                            data=cn[:].rearrange("p (st y) -> p st y", st=2))
                        tcn = gtile("tcn")
                        nc.scalar.activation(out=tcn[:], in_=cn[:], func=AF.Tanh)
                        t3 = gtile("t3")
                        nc.vector.tensor_tensor(out=t3[:], in0=sig_o[:],
                                                in1=tcn[:], op=OP.mult)
                        # masked h -> hstore (strided, f16)
                        hs_ap = hstore[:].rearrange(
                            "p (st b sx) -> p st b sx", st=2, b=BS)[
                            :, :, gofs:gofs + GB, s]
                        nc.vector.tensor_tensor(
                            out=hs_ap, in0=t3[:].rearrange(
                                "p (st y) -> p st y", st=2),
                            in1=mb, op=OP.mult)
                        # predicated update of carry h
                        nc.vector.copy_predicated(
                            out=hg.rearrange("p (st y) -> p st y", st=2),
                            mask=mbi, data=hs_ap)

                        # online score matmuls (accumulate onto zero-fill)
                        for kc in range(2):
                            nc.tensor.matmul(
                                out=score_ps[32 * g:32 * g + GB, s:s + 1],
                                lhsT=hstore[:].rearrange(
                                    "p (st b sx) -> p st b sx", st=2, b=BS)[
                                    :, kc, gofs:gofs + GB, s],
                                rhs=watt_sb[:, kc:kc + 1],
                                start=False, stop=(kc == 1))

            # ---------- end phase ----------
            # own scores -> transposed + b_att -> sbuf; write cc_in
            s_own = wp.tile([64, t_steps], F32)
            for g in range(NG):
                nc.scalar.activation(out=s_own[32 * g:32 * g + GB, :],
                                     in_=score_ps[32 * g:32 * g + GB, :],
                                     func=AF.Copy)
            ownT = wp.tile([128, nsc * BS], F32)
            for c in range(nsc):
                stp = pscratch([128, 64], F32)
                nc.tensor.transpose(out=stp[:, 0:64],
                                    in_=s_own[:, c * 128:(c + 1) * 128],
                                    identity=ident32[0:64, 0:64])
                nc.scalar.activation(
                    out=ownT[:, c * BS:(c + 1) * BS].rearrange(
                        "p (g r) -> p g r", g=NG),
                    in_=stp[:].rearrange("p (g r) -> p g r", g=2)[:, :, 0:GB],
                    func=AF.Identity, bias=battb_sb[:, 0:1])
            nc.sync.dma_start(
                out=cc_in.rearrange("(c p) b -> p c b", p=128),
                in_=ownT[:].rearrange("p (c b) -> p c b", c=nsc))
            nc.gpsimd.collective_compute(
                kind="AllGather", op=OP.bypass,
                replica_groups=[[0, 4], [1, 5], [2, 6], [3, 7]],
                ins=[cc_in[:]], outs=[cc_gather[:]])
            otherT = wp.tile([128, nsc * BS], F32)
            for c in range(nsc):
                nc.gpsimd.indirect_dma_start(
                    out=otherT[:, c * BS:(c + 1) * BS], out_offset=None,
                    in_=cc_gather[:],
                    in_offset=bass.IndirectOffsetOnAxis(
                        ap=permidx_sb[:, c:c + 1], axis=0))

            sT = wp.tile([128, nsc * BS], F32)
            nc.vector.tensor_tensor(out=sT[:], in0=ownT[:], in1=otherT[:],
                                    op=OP.add)
            nc.scalar.activation(out=sT[:], in_=sT[:], func=AF.Tanh)
            expm = wp.tile([128, nsc * BS], F32)
            nc.scalar.activation(out=expm[:], in_=sT[:], func=AF.Exp)
            nc.vector.tensor_tensor(out=expm[:], in0=expm[:], in1=maskT_sb[:],
                                    op=OP.mult)
            expm16 = wp.tile([128, nsc * BS], F16)
            nc.vector.tensor_copy(out=expm16[:], in_=expm[:])
            den_ps = pscratch([1, BS], F32)
            for c in range(nsc):
                nc.tensor.matmul(out=den_ps[:], lhsT=ones16[:, 0:1],
                                 rhs=expm16[:, c * BS:(c + 1) * BS],
                                 start=(c == 0), stop=(c == nsc - 1))
            rden = wp.tile([1, BS], F32)
            nc.vector.reciprocal(out=rden[:], in_=den_ps[:])
            rden16 = wp.tile([1, BS], F16)
            nc.vector.tensor_copy(out=rden16[:], in_=rden[:])
            rden_bc = pscratch([128, BS], F32)
            nc.tensor.matmul(out=rden_bc[:], lhsT=onesrow[:, 0:128],
                             rhs=rden16[:], start=True, stop=True)
            att16 = wp.tile([128, nsc * BS], F16)
            nc.vector.tensor_tensor(
                out=att16[:].rearrange("p (c b) -> p c b", c=nsc),
                in0=expm[:].rearrange("p (c b) -> p c b", c=nsc),
                in1=rden_bc[:, None, :].to_broadcast([128, nsc, BS]),
                op=OP.mult)

            # bridge + attention pooling: 4 samples per psum batch
            rep_ps = psg.tile([128, 2 * BS], F32, name="rep_ps", tag="pw0")
            nc.tensor.matmul(out=rep_ps[:], lhsT=zerorow[:],
                             rhs=onesrow[:, 0:2 * BS], start=True, stop=False)
            for bq in range(BS // 4):
                for sc in range(nsc):
                    btp = pscratch([128, 4 * 256], F16)
                    for j in range(4):
                        b = bq * 4 + j
                        for st in range(2):
                            hcol = st * BS * t_steps + b * t_steps + sc * 128
                            nc.tensor.transpose(
                                out=btp[:, j * 256 + st * 128:
                                        j * 256 + (st + 1) * 128],
                                in_=hstore[:, hcol:hcol + 128],
                                identity=ident16[:])
                    hsub = wp.tile([128, 4 * 256], F16, tag="hsub", bufs=2)
                    nc.scalar.activation(out=hsub[:], in_=btp[:], func=AF.Copy)
                    for j in range(4):
                        b = bq * 4 + j
                        for mt in range(2):
                            nc.tensor.matmul(
                                out=rep_ps[:, mt * BS + b: mt * BS + b + 1],
                                lhsT=hsub[:, j * 256 + mt * 128:
                                          j * 256 + (mt + 1) * 128],
                                rhs=att16[:, sc * BS + b: sc * BS + b + 1],
                                start=False, stop=(sc == nsc - 1))

            rep16 = wp.tile([128, 2 * BS], F16)
            nc.scalar.activation(out=rep16[:], in_=rep_ps[:], func=AF.Copy)
            if debug:
                nc.sync.dma_start(out=dbg_hst[:], in_=hstore[:])
                nc.sync.dma_start(out=dbg_sT[:], in_=sT[:])
                nc.sync.dma_start(out=dbg_att[:], in_=att16[:])
                nc.sync.dma_start(out=dbg_rep[:], in_=rep16[:])
                nc.sync.dma_start(out=dbg_ownT[:], in_=ownT[:])
                nc.sync.dma_start(out=dbg_other[:], in_=otherT[:])
            op_ps = pscratch([OUT, BS], F32)
            for kc in range(2):
                nc.tensor.matmul(out=op_ps[:],
                                 lhsT=wotT_sb[:, kc * OUT:(kc + 1) * OUT],
                                 rhs=rep16[:, kc * BS:(kc + 1) * BS],
                                 start=(kc == 0), stop=(kc == 1))
            outp_sb = wp.tile([OUT, BS], F32)
            nc.scalar.activation(out=outp_sb[:], in_=op_ps[:],
                                 func=AF.Identity, bias=boutc_sb[:, 0:1])
            nc.sync.dma_start(out=outp[:], in_=outp_sb[:])

    nc.compile()
    return nc


def prep_inputs(x, lengths, tfidf, emb_table, Wih_f, Whh_f, bih_f, bhh_f,
                Wih_b, Whh_b, bih_b, bhh_b, w_att, b_att, W_out, b_out,
                t_steps=T):
    """Host-side shard/remap/cast. Returns list of 8 in_maps."""
    x = np.asarray(x)
    lengths = np.asarray(lengths)
    tfidf = np.asarray(tfidf, dtype=np.float32)
    emb_table = np.ascontiguousarray(np.asarray(emb_table, np.float32))
    wn = tfidf / tfidf.sum(axis=1, keepdims=True)
    ngt = t_steps * BS // 128
    nsc = t_steps // 128

    per_dir = {
        0: (Wih_f, Whh_f, bih_f, bhh_f, w_att[:H], W_out[:, :H], True),
        1: (Wih_b, Whh_b, bih_b, bhh_b, w_att[H:], W_out[:, H:], False),
    }
    in_maps = []
    for core in range(8):
        d = 0 if core < 4 else 1
        sh = core % 4
        bsl = slice(32 * sh, 32 * sh + 32)
        Wih, Whh, bih, bhh, wa, Wo, is_fwd = per_dir[d]
        Wih, Whh = np.asarray(Wih, np.float32), np.asarray(Whh, np.float32)

        xs = x[bsl, :t_steps]                      # [32, t]
        ws = wn[bsl, :t_steps]
        ls = lengths[bsl]
        tmask = (np.arange(t_steps)[None, :] < ls[:, None])  # [32, t] t-space
        if not is_fwd:
            xs = xs[:, ::-1]
            ws = ws[:, ::-1]
            tmask = tmask[:, ::-1]
        idx_flat = xs.T.reshape(-1).astype(np.int32)         # s-major [t*32]
        wtok_flat = ws.T.reshape(-1).astype(np.float32)
        idx_sw = idx_flat.reshape(ngt, 128).T.copy()         # [128, ngt]
        wtok_sw = wtok_flat.reshape(ngt, 128).T.copy()
        maskT = np.ascontiguousarray(tmask.T.astype(np.float32))  # [t, 32]

        s_arr = np.arange(t_steps)
        base = t_steps if is_fwd else 0
        perm = (base + t_steps - 1 - s_arr).astype(np.int32)
        permidx = perm.reshape(nsc, 128).T.copy()            # [128, nsc]

        bias = np.asarray(bih, np.float32) + np.asarray(bhh, np.float32)
        bias2 = np.transpose(bias.reshape(4, 2, 128), (1, 0, 2)).reshape(
            2, H4 // 2)
        in_maps.append({
            "emb": emb_table,
            "idx_sw": np.ascontiguousarray(idx_sw),
            "wtok_sw": np.ascontiguousarray(wtok_sw),
            "maskT": maskT,
            "wihT": np.ascontiguousarray(Wih.T.astype(np.float16)),
            "whhT": np.ascontiguousarray(Whh.T.astype(np.float16)),
            "bias2": np.ascontiguousarray(bias2.astype(np.float16)),
            "sel2_in": np.kron(np.eye(2), np.ones((1, W * BS))).astype(
                np.float16),
            "watt": np.ascontiguousarray(
                np.asarray(wa, np.float32).reshape(2, 128).T
                .astype(np.float16)),
            "battb": np.full((128, 1), float(b_att) if is_fwd else 0.0,
                             np.float32),
            "maskwin": np.ascontiguousarray(
                maskT.reshape(1, -1).astype(np.float16)),
            "permidx": np.ascontiguousarray(permidx),
            "wotT": np.ascontiguousarray(
                np.asarray(Wo, np.float32).T.astype(np.float16)),
            "boutc": (np.asarray(b_out, np.float32).reshape(OUT, 1)
                      if is_fwd else np.zeros((OUT, 1), np.float32)),
        })
    return in_maps


def kernel(**inputs):
    t_steps = T
    if t_steps not in _NC_CACHE:
        _NC_CACHE[t_steps] = build(t_steps)
    nc = _NC_CACHE[t_steps]
    in_maps = prep_inputs(**inputs, t_steps=t_steps)
    res = run_bass_kernel_spmd(nc, in_maps, core_ids=list(range(8)))
    out = np.zeros((128, OUT), np.float32)
    for sh in range(4):
        part = res.results[sh]["outp"] + res.results[sh + 4]["outp"]
        out[32 * sh:32 * sh + 32, :] = part.T
    return out
